# revision 4
# baseline (speedup 1.0000x reference)
"""v5: fp16-pipeline SNN kernel for 8 trn2 cores (pure data parallel).

Per step t (threshold 1, decay beta, spike signs in {-1,+1}):
  psum1 = xh @ W1h (+ones*-1/2) + xh @ W1l + xl @ W1h + s1 @ (-I/2)   [PE]
  mem1  = beta*mem1 + psum1                       [DVE fp32, in place]
  s1    = Sign(mem1 - 1)                          [ACT, fp16 out]
  psum2 = s1a @ W2a + s1a @ W2la + s1b @ W2b + s1b @ W2lb + s2 @ (-I/2)
  mem2  = beta*mem2 + psum2                       [DVE fp32, ping-pong]
  s2    = Sign(mem2 - 1)                          [ACT, fp8e4 out]
Outputs per t: spk = s2 (fp8, host maps (s+1)/2), mem2 f32 (exact).

All matmul operands fp16 (fp8 for the s2/r2 reset pass, +-1/-0.5 exact);
x and W are hi/lo fp16 splits, exact to ~2^-22 — total rel err ~2e-4.
The bf16 hi/lo variant of this scheme measured 2^-17-level (v4); fp16
is strictly better at the same pass count.

Layout per core: 2 slabs x 21 batch-lanes x 2978 columns (BC=125,076,
BPAD=1,000,608 — minimal padding; column groups 1024/1024/930).
  x DRAM [T, 84, slab*2*2978] fp16: cols = slab*5956 + hilo*2978 + c
  s1/m1 per slab [106/105, 2978]; s2/m2 pair-packed [126, 2978], 2 bufs
  (ping-pong so output DMAs never stall the recurrence).
Engine use: PE 13 passes/step; DVE 9 STT/step + ACT 9 Sign/step are the
co-bottleneck (~9.5us/step); x loads on qSP HWDGE, outputs on qACT HWDGE.
"""

import numpy as np
from contextlib import ExitStack
from concurrent.futures import ThreadPoolExecutor

T = 10
NI, NH, NO = 4, 5, 3
BETA = 0.95
THR = 1.0
B_FULL = 1_000_000
NCORES = 8

NBL = 21
NSLAB = 2
NCOLS = 2978
NPB = 1024          # column group width (psum tile)
BC = NSLAB * NBL * NCOLS   # 125,076
BPAD = BC * NCORES         # 1,000,608

XR = NBL * NI       # 84
M1 = NBL * NH       # 105
M2P = 2 * NBL * NO  # 126

HALF = 0.5

bass_mult = None
bass_add = None


def _init_ops():
    global bass_mult, bass_add
    import concourse.mybir as mybir
    bass_mult = mybir.AluOpType.mult
    bass_add = mybir.AluOpType.add


def f16_split(a):
    hi = a.astype(np.float16)
    lo = (a.astype(np.float32) - hi.astype(np.float32)).astype(np.float16)
    return hi, lo


def make_weights(w1, w2):
    w1 = np.asarray(w1, np.float32)
    w2 = np.asarray(w2, np.float32)
    f16 = np.float16
    # W1 [85, 128]: rows (bl,i) -> w1[h,i]; ones row -> -1/2 (exact)
    w1f = np.zeros((XR + 1, 128), np.float32)
    for bl in range(NBL):
        for i in range(NI):
            for h in range(NH):
                w1f[4 * bl + i, 5 * bl + h] = w1[h, i]
    w1f[XR, 0:M1] = -HALF
    w1h, w1l = f16_split(w1f)
    # R1 [105, 128]: -I/2 exact
    r1 = np.zeros((M1, 128), np.float32)
    r1[:, 0:M1] = -HALF * np.eye(M1)
    r1 = r1.astype(f16)
    # W2 per slab [106, 126] zero-block packed; consts folded in ones row
    w2f = np.zeros((2, M1 + 1, M2P), np.float32)
    for s in range(2):
        for bl in range(NBL):
            for h in range(NH):
                for o in range(NO):
                    w2f[s, 5 * bl + h, 63 * s + 3 * bl + o] = w2[o, h] / 2.0
        for bl in range(NBL):
            for o in range(NO):
                w2f[s, M1, 63 * s + 3 * bl + o] = (
                    w2[o].astype(np.float64).sum() / 2.0 - HALF
                )
    w2h, w2l = f16_split(w2f)
    # R2 [126, 126]: -I/2 (exact in fp8e4m3)
    r2 = -HALF * np.eye(M2P, dtype=np.float32)
    return (w1h, w1l), r1, (w2h, w2l), r2


def _split_multi_waits(nc):
    """Walrus accepts only ONE sync-wait per compute instruction; hoist
    extras onto pure-sync EventSemaphore instructions."""
    import concourse.mybir as mybir

    for f in nc.m.functions:
        for blk in f.blocks:
            out = []
            for ins in blk.instructions:
                si = ins.sync_info
                if (
                    si is not None
                    and len(si.on_wait) > 1
                    and not isinstance(ins, mybir.InstEventSemaphore)
                ):
                    waits = list(si.on_wait)
                    for j, w in enumerate(waits[:-1]):
                        out.append(
                            mybir.InstEventSemaphore(
                                name=f"{ins.name}-ws{j}",
                                engine=ins.engine,
                                ins=[],
                                outs=[],
                                sync_info=mybir.SyncInfo(
                                    on_wait=[w], on_update=[]
                                ),
                            )
                        )
                    ins.sync_info = mybir.SyncInfo(
                        on_wait=[waits[-1]], on_update=list(si.on_update)
                    )
                out.append(ins)
            blk.instructions = out
    return nc


def build_nc(split_waits=True, mm_chunk=512, reps=1):
    _init_ops()
    import concourse.bass as bass
    import concourse.mybir as mybir
    from concourse.tile import TileContext

    f32 = mybir.dt.float32
    f16 = mybir.dt.float16
    f8 = mybir.dt.float8e4
    Act = mybir.ActivationFunctionType

    groups = []
    c0 = 0
    while c0 < NCOLS:
        groups.append((c0, min(NPB, NCOLS - c0)))
        c0 += NPB
    SLABW = 2 * NCOLS
    XW = NSLAB * SLABW

    nc = bass.Bass()
    xd = nc.declare_dram_parameter("xd", [T, XR, XW], f16, isOutput=False)
    w1h_d = nc.declare_dram_parameter("w1h", [XR + 1, 128], f16, isOutput=False)
    w1l_d = nc.declare_dram_parameter("w1l", [XR + 1, 128], f16, isOutput=False)
    r1_d = nc.declare_dram_parameter("r1", [M1, 128], f16, isOutput=False)
    w2ha_d = nc.declare_dram_parameter("w2ha", [M1 + 1, M2P], f16, isOutput=False)
    w2hb_d = nc.declare_dram_parameter("w2hb", [M1 + 1, M2P], f16, isOutput=False)
    w2la_d = nc.declare_dram_parameter("w2la", [M1 + 1, M2P], f16, isOutput=False)
    w2lb_d = nc.declare_dram_parameter("w2lb", [M1 + 1, M2P], f16, isOutput=False)
    r2_d = nc.declare_dram_parameter("r2", [M2P, M2P], f8, isOutput=False)
    ones_d = nc.declare_dram_parameter("ones", [1, NCOLS], f16, isOutput=False)
    spk_d = nc.declare_dram_parameter("spk2", [T, M2P, NCOLS], f8, isOutput=True)
    mem_d = nc.declare_dram_parameter("mem2", [T, M2P, NCOLS], f32, isOutput=True)

    with ExitStack() as ctx:
        tc = ctx.enter_context(TileContext(nc))
        wp = ctx.enter_context(tc.tile_pool(name="wp", bufs=1))
        st = ctx.enter_context(tc.tile_pool(name="st", bufs=1))
        xp = ctx.enter_context(tc.tile_pool(name="xp", bufs=1))
        ps = ctx.enter_context(tc.tile_pool(name="ps", bufs=2, space="PSUM"))

        negone = wp.tile([128, 1], f32, tag="negone")
        nc.vector.memset(negone[:], -1.0)
        w1h = wp.tile([XR + 1, 128], f16, tag="w1h")
        w1l = wp.tile([XR + 1, 128], f16, tag="w1l")
        r1 = wp.tile([M1, 128], f16, tag="r1")
        w2ha = wp.tile([M1 + 1, M2P], f16, tag="w2ha")
        w2hb = wp.tile([M1 + 1, M2P], f16, tag="w2hb")
        w2la = wp.tile([M1 + 1, M2P], f16, tag="w2la")
        w2lb = wp.tile([M1 + 1, M2P], f16, tag="w2lb")
        r2 = wp.tile([M2P, M2P], f8, tag="r2")
        for tl, dr in ((w1h, w1h_d), (w1l, w1l_d), (r1, r1_d),
                       (w2ha, w2ha_d), (w2hb, w2hb_d), (w2la, w2la_d),
                       (w2lb, w2lb_d), (r2, r2_d)):
            nc.sync.dma_start(tl[:], dr[:])

        # x ring: [85, XW] fp16, row 84 = ones (set once per buffer)
        xts = [xp.tile([XR + 1, XW], f16, tag=f"x_{r}", name=f"x_{r}")
               for r in range(3)]
        for r in range(3):
            for s in range(NSLAB):
                nc.sync.dma_start(
                    xts[r][XR: XR + 1, s * SLABW: s * SLABW + NCOLS],
                    ones_d[:, :],
                )

        # persistent state
        s1t = [st.tile([M1 + 1, NCOLS], f16, tag=f"s1_{s}", name=f"s1_{s}")
               for s in range(NSLAB)]
        m1t = [st.tile([M1, NCOLS], f32, tag=f"m1_{s}", name=f"m1_{s}")
               for s in range(NSLAB)]
        s2t = [st.tile([M2P, NCOLS], f8, tag=f"s2_{p}", name=f"s2_{p}")
               for p in range(2)]
        m2t = [st.tile([M2P, NCOLS], f32, tag=f"m2_{p}", name=f"m2_{p}")
               for p in range(2)]

        for s in range(NSLAB):
            nc.gpsimd.memset(s1t[s][0:M1, :], -1.0)
            nc.gpsimd.memset(m1t[s][:], 0.0)
            nc.sync.dma_start(s1t[s][M1: M1 + 1, :], ones_d[:, :])
        for p in range(2):
            nc.gpsimd.memset(s2t[p][:], -1.0)
            nc.gpsimd.memset(m2t[p][:], 0.0)

        def mm(out_ap, w_ap, rhs_ap, start, stop):
            n = out_ap.shape[-1]
            o = 0
            while o < n:
                k = min(mm_chunk, n - o)
                nc.tensor.matmul(
                    out_ap[:, o: o + k], w_ap, rhs_ap[:, o: o + k],
                    start=start, stop=stop,
                )
                o += k

        for rt in range(reps * T):
            t = rt % T
            xt = xts[rt % 3]
            nc.sync.dma_start(xt[0:XR, :], xd[t])
            cur, prv = rt % 2, (rt + 1) % 2
            for g, (c0, n) in enumerate(groups):
                cs = slice(c0, c0 + n)
                for s in range(NSLAB):
                    hb = s * SLABW + c0          # hi block cols
                    lb = s * SLABW + NCOLS + c0  # lo block cols
                    ps1 = ps.tile([128, NPB], f32, tag="ps1",
                                  name=f"ps1_{rt}_{g}_{s}")
                    mm(ps1[:, 0:n], w1h[:], xt[:, hb: hb + n],
                       start=True, stop=False)
                    mm(ps1[:, 0:n], w1l[0:XR, :], xt[0:XR, hb: hb + n],
                       start=False, stop=False)
                    mm(ps1[:, 0:n], w1h[0:XR, :], xt[0:XR, lb: lb + n],
                       start=False, stop=False)
                    mm(ps1[:, 0:n], r1[:], s1t[s][0:M1, cs],
                       start=False, stop=True)
                    nc.vector.scalar_tensor_tensor(
                        m1t[s][:, cs], m1t[s][:, cs], BETA,
                        ps1[0:M1, 0:n], bass_mult, bass_add,
                    )
                    nc.scalar.activation(
                        s1t[s][0:M1, cs], m1t[s][:, cs],
                        Act.Sign, bias=negone[0:M1, :],
                    )
                ps2 = ps.tile([M2P, NPB], f32, tag="ps2", name=f"ps2_{rt}_{g}")
                mm(ps2[:, 0:n], w2ha[:], s1t[0][:, cs], start=True, stop=False)
                mm(ps2[:, 0:n], w2la[:], s1t[0][:, cs], start=False, stop=False)
                mm(ps2[:, 0:n], w2hb[:], s1t[1][:, cs], start=False, stop=False)
                mm(ps2[:, 0:n], w2lb[:], s1t[1][:, cs], start=False, stop=False)
                mm(ps2[:, 0:n], r2[:], s2t[prv][:, cs], start=False, stop=True)
                nc.vector.scalar_tensor_tensor(
                    m2t[cur][:, cs], m2t[prv][:, cs], BETA, ps2[:, 0:n],
                    bass_mult, bass_add,
                )
                nc.scalar.activation(
                    s2t[cur][:, cs], m2t[cur][:, cs], Act.Sign,
                    bias=negone[0:M2P, :],
                )
            nc.scalar.dma_start(spk_d[t], s2t[cur][:, :])
            nc.scalar.dma_start(mem_d[t], m2t[cur][:, :])

    if split_waits:
        _split_multi_waits(nc)
    return nc


def prep_core_x(xpad, c):
    xc = xpad[:, c * BC: (c + 1) * BC, :].reshape(T, NSLAB, NBL, NCOLS, NI)
    xc = np.ascontiguousarray(xc.transpose(0, 2, 4, 1, 3))  # t, bl, i, s, col
    xc = xc.reshape(T, XR, NSLAB, NCOLS)
    xh = xc.astype(np.float16)
    xl = (xc - xh.astype(np.float32)).astype(np.float16)
    out = np.empty((T, XR, NSLAB, 2, NCOLS), np.float16)
    out[:, :, :, 0, :] = xh
    out[:, :, :, 1, :] = xl
    return out.reshape(T, XR, NSLAB * 2 * NCOLS)


def unpack_outputs(res_c):
    s2 = res_c["spk2"].astype(np.float32)   # [T, 126, NCOLS] in {-1,+1}
    m2 = res_c["mem2"].astype(np.float32)
    out_s = np.empty((T, BC, NO), np.float32)
    out_m = np.empty((T, BC, NO), np.float32)
    v_s = out_s.reshape(T, NSLAB, NBL, NCOLS, NO)
    v_m = out_m.reshape(T, NSLAB, NBL, NCOLS, NO)
    for s in range(NSLAB):
        rows = slice(63 * s, 63 * s + 63)
        a = s2[:, rows, :].reshape(T, NBL, NO, NCOLS).transpose(0, 1, 3, 2)
        b = m2[:, rows, :].reshape(T, NBL, NO, NCOLS).transpose(0, 1, 3, 2)
        v_s[:, s] = (a + 1.0) * 0.5
        v_m[:, s] = b
    return out_s, out_m


def make_in_maps(x, w1, w2):
    import concourse.mybir as mybir
    (w1h, w1l), r1, (w2h, w2l), r2 = make_weights(w1, w2)
    r2 = r2.astype(mybir.dt.np(mybir.dt.float8e4))
    xpad = np.zeros((T, BPAD, NI), dtype=np.float32)
    xpad[:, :B_FULL] = np.asarray(x, np.float32)
    with ThreadPoolExecutor(8) as ex:
        xs = list(ex.map(lambda c: prep_core_x(xpad, c), range(NCORES)))
    onesv = np.ones((1, NCOLS), np.float16)
    return [
        {"xd": xs[c], "w1h": w1h, "w1l": w1l, "r1": r1,
         "w2ha": w2h[0], "w2hb": w2h[1], "w2la": w2l[0], "w2lb": w2l[1],
         "r2": r2, "ones": onesv}
        for c in range(NCORES)
    ]


def kernel(**inputs):
    x = np.asarray(inputs["x"], dtype=np.float32)
    w1 = np.asarray(inputs["w1"], dtype=np.float32)
    w2 = np.asarray(inputs["w2"], dtype=np.float32)

    from concourse.bass_utils import run_bass_kernel_spmd

    nc = build_nc()
    in_maps = make_in_maps(x, w1, w2)

    res = run_bass_kernel_spmd(nc, in_maps, list(range(NCORES))).results

    spk2 = np.empty((T, BPAD, NO), dtype=np.float32)
    mem2 = np.empty((T, BPAD, NO), dtype=np.float32)

    def fill(c):
        s, m = unpack_outputs(res[c])
        spk2[:, c * BC: (c + 1) * BC] = s
        mem2[:, c * BC: (c + 1) * BC] = m

    with ThreadPoolExecutor(8) as ex:
        list(ex.map(fill, range(NCORES)))
    return spk2[:, :B_FULL], mem2[:, :B_FULL]
